# revision 4
# baseline (speedup 1.0000x reference)
"""Single-head attention kernel for Trainium2, SPMD over 8 NeuronCores.

Problem: x [4,4096,128], Wq/Wk/Wv [128,128] -> y [4,4096,128]
  q = x @ Wq.T ; k = x @ Wk.T ; v = x @ Wv.T
  y = softmax(q k^T / sqrt(128)) v

Sharding: 8 cores = 4 batches x 2 query-halves. Each core receives its
batch's x rotated so that its 2048 queries are rows 0..2047 (attention is
invariant to permuting the key order, so rotation changes nothing) -> all
cores run the identical NEFF with no dynamic offsets and no collectives.

Per-core dataflow (all matmuls bf16 inputs, f32 PSUM accumulation):
  xT  = transpose(x) on PE            [128h, 4096n]
  kT  = Wk @ xT, qT = Wq @ xT[:, :2048]  (W^T as stationary)
  v   = x @ Wv^T  (xT slices stationary)  stored [128n-in-tile, o] per tile
  for each 512-query block:
    for each of 32 key tiles:
      S^T = kT-tile^T @ qT-block      (PE, [128k, 512q], PSUM)
      A^T = exp(S^T * scale)          (ACT, bf16 -> SBUF)
      yT += v-tile^T @ A^T            (PE, [128o, 512q], PSUM accum)
      pair-tree adds of A^T           (DVE, for the softmax denominator)
    l  = colsum(A^T) via strided reduce (DVE) + partition reduce (GpSimd)
    y  = transpose(yT) * (1/l)        (PE transpose + ACT per-partition scale)
"""

import sys

sys.path.insert(0, "/opt/trn_rl_repo")

import numpy as np

import concourse.bass as bass
import concourse.mybir as mybir
from concourse import bacc
from concourse.bass_utils import run_bass_kernel_spmd
from concourse.tile import TileContext
from concourse.masks import make_identity

P = 128
N = 4096  # context length (per batch)
NQ = 2048  # queries per core
H = 128
O = 128
KT = N // P  # 32 key tiles
QBS = 512  # query block size
QB = NQ // QBS  # 4 query blocks
SCALE = 1.0 / np.sqrt(128.0)

F32 = mybir.dt.float32
BF16 = mybir.dt.bfloat16

_cached_nc = None


def build_kernel():
    nc = bacc.Bacc(None, target_bir_lowering=False)

    x_d = nc.declare_dram_parameter("x", [N, H], F32, isOutput=False)
    wq_d = nc.declare_dram_parameter("wq", [H, H], F32, isOutput=False)
    wk_d = nc.declare_dram_parameter("wk", [H, H], F32, isOutput=False)
    wv_d = nc.declare_dram_parameter("wv", [O, H], F32, isOutput=False)
    out_d = nc.declare_dram_parameter("out", [NQ, O], F32, isOutput=True)

    with TileContext(nc) as tc:
        with (
            tc.tile_pool(name="const", bufs=1) as cpool,
            tc.tile_pool(name="big", bufs=1) as big,
        ):
            ident_bf = cpool.tile([P, P], BF16)
            make_identity(nc, ident_bf)
            ident_f32 = cpool.tile([P, P], F32)
            make_identity(nc, ident_f32)

            xT = big.tile([P, N], BF16)  # x^T  [h, n]
            kT = big.tile([P, N], BF16)  # k^T  [d, n]
            qT = big.tile([P, NQ], BF16)  # q^T  [d, nq]
            vsb = big.tile([P, N], BF16)  # v    [n-in-tile, kt*128+o]
            wqT = big.tile([P, P], BF16)
            wkT = big.tile([P, P], BF16)
            wvT = big.tile([P, P], BF16)

            # ---- Stage A: load x (f32->bf16 cast DMA) and transpose on PE
            NCHUNK = 4
            TPC = KT // NCHUNK  # tiles per chunk
            with (
                tc.tile_pool(name="stagea", bufs=2) as sta,
                tc.tile_pool(name="psa", bufs=4, space="PSUM") as psa,
            ):
                # W loads + transposes first (keeps PE busy during x DMA)
                for w_d, wT in ((wq_d, wqT), (wk_d, wkT), (wv_d, wvT)):
                    wst = sta.tile([P, P], BF16, tag="wst")
                    nc.gpsimd.dma_start(out=wst[:], in_=w_d[:])  # cast
                    pw = psa.tile([P, P], BF16, tag="pw")
                    nc.tensor.transpose(pw[:], wst[:], ident_bf[:])
                    nc.vector.tensor_copy(wT[:], pw[:])

                for c in range(NCHUNK):
                    xst = sta.tile([P, TPC, P], BF16, tag="xst")
                    rows = x_d[c * TPC * P : (c + 1) * TPC * P, :]
                    nc.gpsimd.dma_start(
                        out=xst[:], in_=rows.rearrange("(t p) h -> p t h", p=P)
                    )
                    for t in range(TPC):
                        i = c * TPC + t
                        px = psa.tile([P, P], BF16, tag="px")
                        nc.tensor.transpose(px[:], xst[:, t, :], ident_bf[:])
                        nc.vector.tensor_copy(xT[:, i * P : (i + 1) * P], px[:])

            # ---- Stage C: projections
            with tc.tile_pool(name="psc", bufs=4, space="PSUM") as psc:
                for j in range(N // QBS):
                    pk = psc.tile([P, QBS], F32, tag="pk")
                    nc.tensor.matmul(
                        pk[:], wkT[:], xT[:, j * QBS : (j + 1) * QBS],
                        start=True, stop=True,
                    )
                    nc.vector.tensor_copy(kT[:, j * QBS : (j + 1) * QBS], pk[:])
                for j in range(NQ // QBS):
                    pq = psc.tile([P, QBS], F32, tag="pk")
                    nc.tensor.matmul(
                        pq[:], wqT[:], xT[:, j * QBS : (j + 1) * QBS],
                        start=True, stop=True,
                    )
                    nc.vector.tensor_copy(qT[:, j * QBS : (j + 1) * QBS], pq[:])
                for i in range(KT):
                    pv = psc.tile([P, P], F32, tag="pv")
                    nc.tensor.matmul(
                        pv[:], xT[:, i * P : (i + 1) * P], wvT[:],
                        start=True, stop=True,
                    )
                    nc.vector.tensor_copy(vsb[:, i * P : (i + 1) * P], pv[:])

            # ---- Stage D: attention, per query block
            with (
                tc.tile_pool(name="ps_s", bufs=3, space="PSUM") as ps_s,
                tc.tile_pool(name="ps_y", bufs=2, space="PSUM") as ps_y,
                tc.tile_pool(name="ps_sm", bufs=2, space="PSUM") as ps_sm,
                tc.tile_pool(name="apool", bufs=4) as apool,
                tc.tile_pool(name="p1pool", bufs=2) as p1pool,
                tc.tile_pool(name="p2pool", bufs=2) as p2pool,
                tc.tile_pool(name="epi", bufs=2) as epi,
            ):
                for qb in range(QB):
                    q_sl = qT[:, qb * QBS : (qb + 1) * QBS]
                    py = ps_y.tile([P, QBS], F32, tag="py")
                    p2 = p2pool.tile([P, KT // 4, QBS], BF16, tag="p2")

                    # software-pipelined S/exp emission so PE never waits
                    # directly on the exp of the tile it just produced
                    DEPTH = 2
                    s_tiles = {}
                    a_tiles = {}
                    p1_prev = None

                    def emit_s_exp(kt):
                        ps = ps_s.tile([P, QBS], F32, tag="ps")
                        nc.tensor.matmul(
                            ps[:], kT[:, kt * P : (kt + 1) * P], q_sl,
                            start=True, stop=True,
                        )
                        a = apool.tile([P, QBS], BF16, tag="a")
                        nc.scalar.activation(
                            a[:], ps[:], mybir.ActivationFunctionType.Exp,
                            scale=float(SCALE),
                        )
                        s_tiles[kt] = ps
                        a_tiles[kt] = a

                    for kt in range(DEPTH):
                        emit_s_exp(kt)
                    for kt in range(KT):
                        if kt + DEPTH < KT:
                            emit_s_exp(kt + DEPTH)
                        a = a_tiles[kt]
                        nc.tensor.matmul(
                            py[:], vsb[:, kt * P : (kt + 1) * P], a[:],
                            start=(kt == 0), stop=(kt == KT - 1),
                        )
                        # denominator pair-tree on DVE
                        if kt % 2 == 1:
                            p1 = p1pool.tile([P, QBS], BF16, tag="p1")
                            nc.vector.tensor_tensor(
                                p1[:], a_tiles[kt - 1][:], a[:],
                                mybir.AluOpType.add,
                            )
                            if kt % 4 == 1:
                                p1_prev = p1
                            else:
                                nc.vector.tensor_tensor(
                                    p2[:, kt // 4, :], p1_prev[:], p1[:],
                                    mybir.AluOpType.add,
                                )
                            del a_tiles[kt - 1], a_tiles[kt]

                    # epilogue: softmax denominator + normalize + transpose out
                    l_part = epi.tile([P, QBS], F32, tag="l_part")
                    nc.vector.tensor_reduce(
                        l_part[:], p2.rearrange("p t q -> p q t"),
                        axis=mybir.AxisListType.X, op=mybir.AluOpType.add,
                    )
                    l_row = epi.tile([1, QBS], F32, tag="l_row")
                    nc.gpsimd.tensor_reduce(
                        l_row[:], l_part[:],
                        axis=mybir.AxisListType.C, op=mybir.AluOpType.add,
                    )
                    l_rec = epi.tile([1, QBS], F32, tag="l_rec")
                    nc.vector.reciprocal(l_rec[:], l_row[:])

                    y_t = epi.tile([P, QBS], F32, tag="y_t")
                    nc.vector.tensor_copy(y_t[:], py[:])

                    for j in range(QBS // P):
                        plt = ps_sm.tile([P, 1], F32, tag="sm")
                        nc.tensor.transpose(
                            plt[:], l_rec[:, j * P : (j + 1) * P],
                            ident_f32[0:1, 0:1],
                        )
                        lcol = epi.tile([P, 1], F32, tag="lcol")
                        nc.vector.tensor_copy(lcol[:], plt[:])
                        pyt = ps_sm.tile([P, P], F32, tag="sm")
                        nc.tensor.transpose(
                            pyt[:], y_t[:, j * P : (j + 1) * P], ident_f32[:]
                        )
                        yout = epi.tile([P, P], F32, tag="yout")
                        nc.scalar.activation(
                            yout[:], pyt[:], mybir.ActivationFunctionType.Copy,
                            scale=lcol[:, 0:1],
                        )
                        r0 = qb * QBS + j * P
                        nc.sync.dma_start(out=out_d[r0 : r0 + P, :], in_=yout[:])

    nc.compile()
    return nc


def _run(x, Wq, Wk, Wv, **spmd_kwargs):
    global _cached_nc
    if _cached_nc is None:
        _cached_nc = build_kernel()
    nc = _cached_nc

    x = np.asarray(x, dtype=np.float32)
    Wq = np.ascontiguousarray(np.asarray(Wq, dtype=np.float32))
    Wk = np.ascontiguousarray(np.asarray(Wk, dtype=np.float32))
    Wv = np.ascontiguousarray(np.asarray(Wv, dtype=np.float32))

    B = x.shape[0]
    in_maps = []
    for core in range(8):
        b, half = core // 2, core % 2
        xb = x[b]
        if half:
            xb = np.roll(xb, -NQ, axis=0)  # queries -> rows 0..NQ-1
        in_maps.append(
            {"x": np.ascontiguousarray(xb), "wq": Wq, "wk": Wk, "wv": Wv}
        )

    res = run_bass_kernel_spmd(nc, in_maps, core_ids=list(range(8)), **spmd_kwargs)

    y = np.empty((B, N, O), dtype=np.float32)
    for core in range(8):
        b, half = core // 2, core % 2
        y[b, half * NQ : (half + 1) * NQ] = res.results[core]["out"]
    return y, res


def kernel(x, Wq, Wk, Wv):
    y, _ = _run(x, Wq, Wk, Wv)
    return y


if __name__ == "__main__":
    rng = np.random.default_rng(0)
    x = rng.standard_normal((4, N, H), dtype=np.float32)
    Wq = rng.standard_normal((H, H), dtype=np.float32) / np.sqrt(H)
    Wk = rng.standard_normal((H, H), dtype=np.float32) / np.sqrt(H)
    Wv = rng.standard_normal((O, H), dtype=np.float32) / np.sqrt(H)
    y = kernel(x=x, Wq=Wq, Wk=Wk, Wv=Wv)
    print("kernel output", y.shape, y.dtype)


# revision 7
# speedup vs baseline: 3.2411x; 3.2411x over previous
"""Single-head attention kernel for Trainium2, SPMD over 8 NeuronCores.

Problem: x [4,4096,128], Wq/Wk/Wv [128,128] -> y [4,4096,128]
  q = x @ Wq.T ; k = x @ Wk.T ; v = x @ Wv.T
  y = softmax(q k^T / sqrt(128)) v

Sharding: 8 cores = 4 batches x 2 query-halves. Each core receives its
batch's x rotated so that its 2048 queries are rows 0..2047 (attention is
invariant to permuting the key order, so rotation changes nothing) -> all
cores run the identical NEFF with no dynamic offsets and no collectives.

Per-core dataflow (all attention matmuls bf16 inputs, f32 PSUM accum):
  xT = transpose(x) on PE                [128h, 4096n]
  kT = Wk @ xT, qT = Wq @ xT[:, :2048]   (W^T stationary)
  v  = x @ Wv^T                          ([128n-in-tile, o] per tile)
  for each 1024-query block:
    for each of 32 key tiles:
      S^T = kT-tile^T @ qT-block         (PE, 2x N=512 into [128k,1024] PSUM)
      A^T = exp(S^T * scale)             (ACT, one op per 1024, bf16 SBUF)
      yT += v-tile^T @ A^T               (PE, [128o,1024q] PSUM accum)
      3-level bf16 pair-tree of A^T      (DVE, softmax denominator)
    l   = ones^T @ tree-roots            (PE accumulating [1,512]x2 PSUM)
    y   = transpose(yT) * (1/l)          (PE transpose + ACT per-part scale)
"""

import sys

sys.path.insert(0, "/opt/trn_rl_repo")

import numpy as np

import concourse.bass as bass
import concourse.mybir as mybir
from concourse import bacc
from concourse.bass_utils import run_bass_kernel_spmd
from concourse.tile import TileContext
from concourse.masks import make_identity

P = 128
N = 4096  # context length (per batch)
NQ = 2048  # queries per core
H = 128
O = 128
KT = N // P  # 32 key tiles
QBS = 1024  # query block size
QB = NQ // QBS  # 2 query blocks
SCALE = 1.0 / np.sqrt(128.0)

F32 = mybir.dt.float32
BF16 = mybir.dt.bfloat16

_cached_nc = None


def build_kernel():
    nc = bacc.Bacc(None, target_bir_lowering=False)

    x_d = nc.declare_dram_parameter("x", [N, H], F32, isOutput=False)
    wq_d = nc.declare_dram_parameter("wq", [H, H], F32, isOutput=False)
    wk_d = nc.declare_dram_parameter("wk", [H, H], F32, isOutput=False)
    wv_d = nc.declare_dram_parameter("wv", [O, H], F32, isOutput=False)
    out_d = nc.declare_dram_parameter("out", [NQ, O], F32, isOutput=True)

    with TileContext(nc) as tc:
        with (
            tc.tile_pool(name="const", bufs=1) as cpool,
            tc.tile_pool(name="big", bufs=1) as big,
        ):
            ident_bf = cpool.tile([P, P], BF16)
            make_identity(nc, ident_bf)
            ident_f32 = cpool.tile([P, P], F32)
            make_identity(nc, ident_f32)
            ones_bf = cpool.tile([P, 1], BF16)
            nc.gpsimd.memset(ones_bf[:], 1.0)

            xT = big.tile([P, N], BF16)  # x^T  [h, n]
            kT = big.tile([P, N], BF16)  # k^T  [d, n]
            qT = big.tile([P, NQ], BF16)  # q^T  [d, nq]
            vsb = big.tile([P, N], BF16)  # v    [n-in-tile, kt*128+o]
            wqT = big.tile([P, P], BF16)
            wkT = big.tile([P, P], BF16)
            wvT = big.tile([P, P], BF16)

            # ---- Stage A: load x (f32->bf16 cast DMA) and transpose on PE
            NCHUNK = 4
            TPC = KT // NCHUNK  # 8 tiles per chunk
            with (
                tc.tile_pool(name="stagea", bufs=2) as sta,
                tc.tile_pool(name="psa", bufs=3, space="PSUM") as psa,
            ):
                # W loads + transposes first (keeps PE busy during x DMA)
                pw = psa.tile([P, 3 * P], BF16, tag="pw")
                for wi, (w_d, wT) in enumerate(
                    ((wq_d, wqT), (wk_d, wkT), (wv_d, wvT))
                ):
                    wst = sta.tile([P, P], BF16, tag="wst")
                    nc.gpsimd.dma_start(out=wst[:], in_=w_d[:])  # cast
                    nc.tensor.transpose(
                        pw[:, wi * P : (wi + 1) * P], wst[:], ident_bf[:]
                    )
                for wi, wT in enumerate((wqT, wkT, wvT)):
                    nc.vector.tensor_copy(wT[:], pw[:, wi * P : (wi + 1) * P])

                for c in range(NCHUNK):
                    xst = sta.tile([P, TPC, P], BF16, tag="xst")
                    rows = x_d[c * TPC * P : (c + 1) * TPC * P, :]
                    nc.gpsimd.dma_start(
                        out=xst[:], in_=rows.rearrange("(t p) h -> p t h", p=P)
                    )
                    # pack 4 transposes per PSUM tile, copy out 512 at a time
                    for g in range(TPC // 4):
                        px = psa.tile([P, 4 * P], BF16, tag="px")
                        for t4 in range(4):
                            t = g * 4 + t4
                            nc.tensor.transpose(
                                px[:, t4 * P : (t4 + 1) * P],
                                xst[:, t, :],
                                ident_bf[:],
                            )
                        i0 = (c * TPC + g * 4) * P
                        nc.vector.tensor_copy(xT[:, i0 : i0 + 4 * P], px[:])

            # ---- Stage C: projections
            with tc.tile_pool(name="psc", bufs=4, space="PSUM") as psc:
                for j in range(N // 512):
                    pk = psc.tile([P, 512], F32, tag="pk")
                    nc.tensor.matmul(
                        pk[:], wkT[:], xT[:, j * 512 : (j + 1) * 512],
                        start=True, stop=True,
                    )
                    nc.vector.tensor_copy(kT[:, j * 512 : (j + 1) * 512], pk[:])
                for j in range(NQ // 512):
                    pq = psc.tile([P, 512], F32, tag="pk")
                    nc.tensor.matmul(
                        pq[:], wqT[:], xT[:, j * 512 : (j + 1) * 512],
                        start=True, stop=True,
                    )
                    nc.vector.tensor_copy(qT[:, j * 512 : (j + 1) * 512], pq[:])
                for g in range(KT // 4):
                    pv = psc.tile([P, 512], F32, tag="pk")
                    for t4 in range(4):
                        i = g * 4 + t4
                        nc.tensor.matmul(
                            pv[:, t4 * P : (t4 + 1) * P],
                            xT[:, i * P : (i + 1) * P], wvT[:],
                            start=True, stop=True,
                        )
                    nc.vector.tensor_copy(vsb[:, g * 512 : (g + 1) * 512], pv[:])

            # ---- Stage D: attention, per query block
            with (
                tc.tile_pool(name="ps_s", bufs=2, space="PSUM") as ps_s,
                tc.tile_pool(name="ps_y", bufs=1, space="PSUM") as ps_y,
                tc.tile_pool(name="ps_sm", bufs=2, space="PSUM") as ps_sm,
                tc.tile_pool(name="apool", bufs=4) as apool,
                tc.tile_pool(name="tpool", bufs=3) as tpool,
                tc.tile_pool(name="epi", bufs=2) as epi,
            ):
                for qb in range(QB):
                    q0 = qb * QBS
                    py = ps_y.tile([P, QBS], F32, tag="py")

                    # software-pipelined S/exp emission so PE runs ahead of ACT
                    DEPTH = 2
                    a_tiles = {}
                    tree_prev = {1: None, 2: None}
                    roots = []

                    def emit_s_exp(kt):
                        ps = ps_s.tile([P, QBS], F32, tag="ps")
                        for h in range(QBS // 512):
                            nc.tensor.matmul(
                                ps[:, h * 512 : (h + 1) * 512],
                                kT[:, kt * P : (kt + 1) * P],
                                qT[:, q0 + h * 512 : q0 + (h + 1) * 512],
                                start=True, stop=True,
                            )
                        a = apool.tile([P, QBS], BF16, tag="a")
                        nc.scalar.activation(
                            a[:], ps[:], mybir.ActivationFunctionType.Exp,
                            scale=float(SCALE),
                        )
                        a_tiles[kt] = a

                    def tree_add(level, t):
                        out = tpool.tile([P, QBS], BF16, tag=f"t{level}")
                        prev = tree_prev[level]
                        nc.vector.tensor_tensor(
                            out[:], prev[:], t[:], mybir.AluOpType.add
                        )
                        tree_prev[level] = None
                        return out

                    for kt in range(DEPTH):
                        emit_s_exp(kt)
                    for kt in range(KT):
                        if kt + DEPTH < KT:
                            emit_s_exp(kt + DEPTH)
                        a = a_tiles.pop(kt)
                        for h in range(QBS // 512):
                            nc.tensor.matmul(
                                py[:, h * 512 : (h + 1) * 512],
                                vsb[:, kt * P : (kt + 1) * P],
                                a[:, h * 512 : (h + 1) * 512],
                                start=(kt == 0), stop=(kt == KT - 1),
                            )
                        # 3-level pair tree for the softmax denominator
                        if tree_prev[1] is None:
                            tree_prev[1] = a
                        else:
                            p1 = tree_add(1, a)
                            if tree_prev[2] is None:
                                tree_prev[2] = p1
                            else:
                                roots.append(tree_add(2, p1))

                    # l = sum over k of A^T: accumulate ones^T @ root tiles
                    pls = [
                        ps_sm.tile([1, 512], F32, tag="sm", name=f"pl{_h}")
                        for _h in range(QBS // 512)
                    ]
                    nroots = len(roots)
                    for ri, r in enumerate(roots):
                        for h in range(QBS // 512):
                            nc.tensor.matmul(
                                pls[h][:],
                                ones_bf[:],
                                r[:, h * 512 : (h + 1) * 512],
                                start=(ri == 0), stop=(ri == nroots - 1),
                            )
                    l_row = epi.tile([1, QBS], F32, tag="l_row")
                    for h in range(QBS // 512):
                        nc.scalar.activation(
                            l_row[:, h * 512 : (h + 1) * 512], pls[h][:],
                            mybir.ActivationFunctionType.Copy,
                        )

                    y_t = epi.tile([P, QBS], F32, tag="y_t")
                    nc.vector.tensor_copy(y_t[:], py[:])

                    for j in range(QBS // P):
                        psm = ps_sm.tile([P, P + 1], F32, tag="sm")
                        nc.tensor.transpose(
                            psm[:, 0:1],
                            l_row[:, j * P : (j + 1) * P],
                            ident_f32[0:1, 0:1],
                        )
                        nc.tensor.transpose(
                            psm[:, 1 : P + 1], y_t[:, j * P : (j + 1) * P],
                            ident_f32[:],
                        )
                        lcol = epi.tile([P, 1], F32, tag="lcol")
                        nc.vector.reciprocal(lcol[:], psm[:, 0:1])
                        yout = epi.tile([P, P], F32, tag="yout")
                        nc.scalar.activation(
                            yout[:], psm[:, 1 : P + 1],
                            mybir.ActivationFunctionType.Copy,
                            scale=lcol[:, 0:1],
                        )
                        r0 = q0 + j * P
                        nc.sync.dma_start(out=out_d[r0 : r0 + P, :], in_=yout[:])

    nc.compile()
    return nc


def _run(x, Wq, Wk, Wv, **spmd_kwargs):
    global _cached_nc
    if _cached_nc is None:
        _cached_nc = build_kernel()
    nc = _cached_nc

    x = np.asarray(x, dtype=np.float32)
    Wq = np.ascontiguousarray(np.asarray(Wq, dtype=np.float32))
    Wk = np.ascontiguousarray(np.asarray(Wk, dtype=np.float32))
    Wv = np.ascontiguousarray(np.asarray(Wv, dtype=np.float32))

    B = x.shape[0]
    in_maps = []
    for core in range(8):
        b, half = core // 2, core % 2
        xb = x[b]
        if half:
            xb = np.roll(xb, -NQ, axis=0)  # queries -> rows 0..NQ-1
        in_maps.append(
            {"x": np.ascontiguousarray(xb), "wq": Wq, "wk": Wk, "wv": Wv}
        )

    res = run_bass_kernel_spmd(nc, in_maps, core_ids=list(range(8)), **spmd_kwargs)

    y = np.empty((B, N, O), dtype=np.float32)
    for core in range(8):
        b, half = core // 2, core % 2
        y[b, half * NQ : (half + 1) * NQ] = res.results[core]["out"]
    return y, res


def kernel(x, Wq, Wk, Wv):
    y, _ = _run(x, Wq, Wk, Wv)
    return y


if __name__ == "__main__":
    rng = np.random.default_rng(0)
    x = rng.standard_normal((4, N, H), dtype=np.float32)
    Wq = rng.standard_normal((H, H), dtype=np.float32) / np.sqrt(H)
    Wk = rng.standard_normal((H, H), dtype=np.float32) / np.sqrt(H)
    Wv = rng.standard_normal((O, H), dtype=np.float32) / np.sqrt(H)
    y = kernel(x=x, Wq=Wq, Wk=Wk, Wv=Wv)
    print("kernel output", y.shape, y.dtype)
